# revision 4
# baseline (speedup 1.0000x reference)
"""DLRM tower (embedding_lookup) Trainium2 Bass kernel.

Strategy: pure data parallelism over 8 NeuronCores. Each core processes
B/8 = 2048 samples with the full embedding tables replicated in its DRAM
(staging is untimed). No collectives.

Per-core pipeline (all fp32):
  - bottom MLP computed transposed: H^T = W^T @ X^T, so activations live as
    [feature, sample] with features on partitions,
  - ALL embedding row offsets preloaded into SBUF in one DMA, then one
    indirect-DMA gather per 128-sample tile fetches 26*128 rows (512B each)
    as [sample, table, 128],
  - PE transposes flip each gathered 128x128 block into T^T[d, s*27+f];
    PSUM->SBUF copies alternate between the vector and scalar engines,
  - per-4-sample gram matmuls [108x108] give all pairwise dots,
  - diagonal 27x27 blocks are extracted (alternating vector/scalar) and
    re-tiled via SBUF->SBUF DMA into k-tiles with pair-index on partitions,
  - projection contracts the FULL 27x27 gram against a host-built
    symmetric-expanded Wp (0.5 on off-diagonal, 0 on diagonal), which is
    exactly equivalent to the upper-triangle flatten of the reference,
  - output written transposed [512, 2048]; host transposes back.
"""

from contextlib import ExitStack

import numpy as np

import concourse.bass as bass
import concourse.tile as tile
from concourse import bacc, mybir
from concourse._compat import with_exitstack
from concourse.bass_utils import run_bass_kernel_spmd
from concourse.masks import make_identity

F32 = mybir.dt.float32
I32 = mybir.dt.int32

N_CAT = 26
VOCAB = 50000
D = 128
B = 16384
DENSE = 13
MLP0, MLP1, MLP2 = 512, 256, 128
PROJ = 512
NF = N_CAT + 1              # 27 features entering interaction
GRAM = NF * NF              # 729 full-gram rows
KEXT = MLP2 + GRAM          # 857 projection contraction size
NCORES = 8
BC = B // NCORES            # 2048 samples per core
STRIPES = 4                 # 512-sample stripes per core
SPT = BC // STRIPES         # 512
TILES = 4                   # 128-sample tiles per stripe
TS = 128
NTILES = STRIPES * TILES    # 16
# gram k-tiles: groups of 4 feature-rows (i) -> 108 pair-rows, last has 3 -> 81
GK_ROWS = [108, 108, 108, 108, 108, 108, 81]


@with_exitstack
def _dlrm_kernel(ctx: ExitStack, tc: tile.TileContext,
                 emb, xt, off, w0, w1, w2, wpe, b0, b1, b2, bp, outT):
    nc = tc.nc
    Relu = mybir.ActivationFunctionType.Relu
    Ident = mybir.ActivationFunctionType.Identity
    F32R = mybir.dt.float32r

    def r(ap):
        # PE-native reduced fp32 (FP22): 1 cycle/col when moving N >= 256
        return ap.bitcast(F32R)

    consts = ctx.enter_context(tc.tile_pool(name="consts", bufs=1))
    gp = ctx.enter_context(tc.tile_pool(name="gp", bufs=2))
    ttp = ctx.enter_context(tc.tile_pool(name="ttp", bufs=2))
    zdp = ctx.enter_context(tc.tile_pool(name="zdp", bufs=2))
    h1p = ctx.enter_context(tc.tile_pool(name="h1p", bufs=2))
    h2p = ctx.enter_context(tc.tile_pool(name="h2p", bufs=2))
    motp = ctx.enter_context(tc.tile_pool(name="motp", bufs=2))
    ztp = ctx.enter_context(tc.tile_pool(name="ztp", bufs=2))
    outp = ctx.enter_context(tc.tile_pool(name="outp", bufs=2))
    pm = ctx.enter_context(tc.tile_pool(name="pm", bufs=2, space="PSUM"))
    pt = ctx.enter_context(tc.tile_pool(name="pt", bufs=4, space="PSUM"))
    pg = ctx.enter_context(tc.tile_pool(name="pg", bufs=2, space="PSUM"))

    ident = consts.tile([128, 128], F32)
    make_identity(nc, ident[:])

    w0s = consts.tile([DENSE, MLP0], F32R)
    nc.sync.dma_start(w0s[:], r(w0[:]))
    w1s = consts.tile([128, 4, MLP1], F32)
    for k in range(4):
        nc.sync.dma_start(w1s[:, k, :], w1[k * 128:(k + 1) * 128, :])
    w2s = consts.tile([128, 2, MLP2], F32)
    for k in range(2):
        nc.sync.dma_start(w2s[:, k, :], w2[k * 128:(k + 1) * 128, :])
    # projection weights: k-tile 0 = mlp rows [0:128]; k-tiles 1..7 = gram rows
    wps = consts.tile([128, 8, PROJ], F32R)
    nc.sync.dma_start(wps[:, 0, :], r(wpe[0:128, :]))
    r0 = 128
    for kk in range(7):
        rw0 = GK_ROWS[kk]
        nc.sync.dma_start(wps[0:rw0, kk + 1, :], r(wpe[r0:r0 + rw0, :]))
        r0 += rw0
    b0s = consts.tile([128, 4], F32)
    for m in range(4):
        nc.sync.dma_start(b0s[:, m:m + 1], b0[m * 128:(m + 1) * 128, :])
    b1s = consts.tile([128, 2], F32)
    for m in range(2):
        nc.sync.dma_start(b1s[:, m:m + 1], b1[m * 128:(m + 1) * 128, :])
    b2s = consts.tile([128, 1], F32)
    nc.sync.dma_start(b2s[:], b2[:])
    bps = consts.tile([128, 4], F32)
    for m in range(4):
        nc.sync.dma_start(bps[:, m:m + 1], bp[m * 128:(m + 1) * 128, :])
    xts = consts.tile([DENSE, BC], F32R)
    nc.sync.dma_start(xts[:], r(xt[:]))
    # ALL gather offsets in one load: off[p, kt*26+t]
    offs = consts.tile([128, NTILES * N_CAT], I32)
    nc.sync.dma_start(offs[:], off[:])

    for s in range(STRIPES):
        cs = bass.ds(s * SPT, SPT)
        # ---- bottom MLP (transposed: features on partitions) ----
        h1 = h1p.tile([128, 4, SPT], F32)
        for m in range(4):
            ps = pm.tile([128, SPT], F32)
            nc.tensor.matmul(ps[:], lhsT=w0s[:, m * 128:(m + 1) * 128],
                             rhs=xts[:, cs], start=True, stop=True)
            nc.scalar.activation(h1[:, m, :], ps[:], Relu, bias=b0s[:, m:m + 1])
        h2 = h2p.tile([128, 2, SPT], F32)
        for m in range(2):
            ps = pm.tile([128, SPT], F32)
            for k in range(4):
                nc.tensor.matmul(ps[:], lhsT=w1s[:, k, m * 128:(m + 1) * 128],
                                 rhs=h1[:, k, :], start=(k == 0), stop=(k == 3))
            nc.scalar.activation(h2[:, m, :], ps[:], Relu, bias=b1s[:, m:m + 1])
        mot = motp.tile([128, SPT], F32)
        ps = pm.tile([128, SPT], F32)
        for k in range(2):
            nc.tensor.matmul(ps[:], lhsT=w2s[:, k, :], rhs=h2[:, k, :],
                             start=(k == 0), stop=(k == 1))
        nc.scalar.activation(mot[:], ps[:], Ident, bias=b2s[:, 0:1])

        zt = ztp.tile([108, 7, SPT], F32R)
        for tt in range(TILES):
            kt = s * TILES + tt
            # per-table indirect gathers: g[p, t, :] = emb[offs[p, kt*26+t]]
            g = gp.tile([128, N_CAT, D], F32)
            for t in range(N_CAT):
                c = kt * N_CAT + t
                nc.gpsimd.indirect_dma_start(
                    out=g[:, t, :], out_offset=None, in_=emb[:],
                    in_offset=bass.IndirectOffsetOnAxis(
                        ap=offs[:, c:c + 1], axis=0))
            # T^T layout: [d, s_local, f] with f padded to 32 so gram diag
            # blocks land on 32-aligned PSUM partitions (walrus requirement)
            ttile = ttp.tile([128, TS, 32], F32)
            nc.vector.memset(ttile[:, :, NF:32], 0.0)
            nc.vector.tensor_copy(ttile[:, :, 0], mot[:, tt * TS:(tt + 1) * TS])
            for t in range(N_CAT):
                pst = pt.tile([128, 128], F32)
                nc.tensor.transpose(pst[:], g[:, t, :], ident[:])
                if t % 2 == 0:
                    nc.vector.tensor_copy(ttile[:, :, t + 1], pst[:])
                else:
                    nc.scalar.copy(ttile[:, :, t + 1], pst[:])
            zd = zdp.tile([NF, NF, TS], F32)
            for gi in range(TS // 4):
                pgr = pg.tile([128, 128], F32)
                nc.tensor.matmul(pgr[:], lhsT=ttile[:, gi * 4:(gi + 1) * 4, :],
                                 rhs=ttile[:, gi * 4:(gi + 1) * 4, :],
                                 start=True, stop=True)
                for u in range(4):
                    sl = gi * 4 + u
                    blk = pgr[32 * u:32 * u + 27, 32 * u:32 * u + 27]
                    if u % 2 == 0:
                        nc.vector.tensor_copy(zd[:, :, sl], blk)
                    else:
                        nc.scalar.copy(zd[:, :, sl], blk)
            # re-tile: pair q = i*27+j -> k-tile i//4, row (i%4)*27+j
            for i in range(NF):
                kk = i // 4
                rr = (i % 4) * 27
                nc.sync.dma_start(zt[rr:rr + 27, kk, tt * TS:(tt + 1) * TS],
                                  r(zd[i:i + 1, :, :]))
        # ---- projection ----
        for m in range(4):
            ps = pm.tile([128, SPT], F32)
            nc.tensor.matmul(ps[:], lhsT=wps[:, 0, m * 128:(m + 1) * 128].bitcast(F32),
                             rhs=mot[:], start=True, stop=False)
            for kk in range(7):
                rw = GK_ROWS[kk]
                nc.tensor.matmul(ps[:], lhsT=wps[0:rw, kk + 1, m * 128:(m + 1) * 128],
                                 rhs=zt[0:rw, kk, :], start=False, stop=(kk == 6))
            ot = outp.tile([128, SPT], F32)
            nc.scalar.activation(ot[:], ps[:], Ident, bias=bps[:, m:m + 1])
            nc.sync.dma_start(outT[m * 128:(m + 1) * 128, cs], ot[:])


_PROG = None


def _build_program():
    global _PROG
    if _PROG is not None:
        return _PROG
    nc = bacc.Bacc("TRN2", target_bir_lowering=False, debug=False,
                   enable_asserts=False, num_devices=NCORES)
    emb = nc.dram_tensor("emb", [N_CAT * VOCAB, D], F32, kind="ExternalInput").ap()
    xt = nc.dram_tensor("xt", [DENSE, BC], F32, kind="ExternalInput").ap()
    off = nc.dram_tensor("off", [128, NTILES * N_CAT], I32,
                         kind="ExternalInput").ap()
    w0 = nc.dram_tensor("w0", [DENSE, MLP0], F32, kind="ExternalInput").ap()
    w1 = nc.dram_tensor("w1", [MLP0, MLP1], F32, kind="ExternalInput").ap()
    w2 = nc.dram_tensor("w2", [MLP1, MLP2], F32, kind="ExternalInput").ap()
    wpe = nc.dram_tensor("wpe", [KEXT, PROJ], F32, kind="ExternalInput").ap()
    b0 = nc.dram_tensor("b0", [MLP0, 1], F32, kind="ExternalInput").ap()
    b1 = nc.dram_tensor("b1", [MLP1, 1], F32, kind="ExternalInput").ap()
    b2 = nc.dram_tensor("b2", [MLP2, 1], F32, kind="ExternalInput").ap()
    bp = nc.dram_tensor("bp", [PROJ, 1], F32, kind="ExternalInput").ap()
    outT = nc.dram_tensor("outT", [PROJ, BC], F32, kind="ExternalOutput").ap()
    with tile.TileContext(nc) as tc:
        _dlrm_kernel(tc, emb, xt, off, w0, w1, w2, wpe, b0, b1, b2, bp, outT)
    nc.compile()
    _PROG = nc
    return nc


def _expand_wp(Wp: np.ndarray) -> np.ndarray:
    """[479, 512] -> [857, 512]: full-gram rows, 0.5 off-diag, 0 diag."""
    wpe = np.zeros((KEXT, PROJ), np.float32)
    wpe[:MLP2] = Wp[:MLP2]
    row, col = np.triu_indices(NF, k=1)
    pair_q = {}
    for q, (i, j) in enumerate(zip(row, col)):
        pair_q[(i, j)] = q
    for i in range(NF):
        for j in range(NF):
            if i == j:
                continue
            a, b = (i, j) if i < j else (j, i)
            wpe[MLP2 + i * NF + j] = 0.5 * Wp[MLP2 + pair_q[(a, b)]]
    return wpe


def prepare_in_maps(dense, emb_indices, W0, b0, W1, b1, W2, b2, emb_tables, Wp, bp):
    dense = np.asarray(dense, np.float32)
    emb_indices = np.asarray(emb_indices)
    emb_flat = np.ascontiguousarray(np.asarray(emb_tables, np.float32)
                                    .reshape(N_CAT * VOCAB, D))
    wpe = _expand_wp(np.asarray(Wp, np.float32))
    base = (np.arange(N_CAT, dtype=np.int64) * VOCAB)[:, None]
    gidx = (emb_indices.astype(np.int64) + base).astype(np.int32)  # [26, B]
    common = {
        "emb": emb_flat,
        "w0": np.asarray(W0, np.float32),
        "w1": np.asarray(W1, np.float32),
        "w2": np.asarray(W2, np.float32),
        "wpe": wpe,
        "b0": np.asarray(b0, np.float32).reshape(MLP0, 1),
        "b1": np.asarray(b1, np.float32).reshape(MLP1, 1),
        "b2": np.asarray(b2, np.float32).reshape(MLP2, 1),
        "bp": np.asarray(bp, np.float32).reshape(PROJ, 1),
    }
    in_maps = []
    for c in range(NCORES):
        sl = slice(c * BC, (c + 1) * BC)
        # off[p, kt*26+t] = gidx[t, c*2048 + kt*128 + p]
        oc = gidx[:, sl].reshape(N_CAT, NTILES, TS)  # [t, kt, p]
        off = np.ascontiguousarray(oc.transpose(2, 1, 0)
                                   .reshape(TS, NTILES * N_CAT))
        in_maps.append(dict(common,
                            xt=np.ascontiguousarray(dense[sl].T),
                            off=off))
    return in_maps


def kernel(**inputs) -> np.ndarray:
    nc = _build_program()
    in_maps = prepare_in_maps(**inputs)
    res = run_bass_kernel_spmd(nc, in_maps, list(range(NCORES)))
    out = np.empty((B, PROJ), np.float32)
    for c in range(NCORES):
        out[c * BC:(c + 1) * BC] = res.results[c]["outT"].T
    return out


# revision 6
# speedup vs baseline: 2.0462x; 2.0462x over previous
"""DLRM tower (embedding_lookup) Trainium2 Bass kernel.

Strategy: pure data parallelism over 8 NeuronCores. Each core processes
B/8 = 2048 samples with the full embedding tables replicated in its DRAM
(staging is untimed). No collectives.

Per-core pipeline (all fp32):
  - bottom MLP computed transposed: H^T = W^T @ X^T, so activations live as
    [feature, sample] with features on partitions,
  - ALL embedding row offsets preloaded into SBUF in one DMA, then one
    indirect-DMA gather per 128-sample tile fetches 26*128 rows (512B each)
    as [sample, table, 128],
  - PE transposes flip each gathered 128x128 block into T^T[d, s*27+f];
    PSUM->SBUF copies alternate between the vector and scalar engines,
  - per-4-sample gram matmuls [108x108] give all pairwise dots,
  - diagonal 27x27 blocks are extracted (3:1 vector:scalar split) and
    re-tiled via SBUF->SBUF DMA into k-tiles with pair-index on partitions,
  - projection contracts the FULL 27x27 gram against a host-built
    symmetric-expanded Wp (0.5 on off-diagonal, 0 on diagonal), which is
    exactly equivalent to the upper-triangle flatten of the reference,
  - output written transposed [512, 2048]; host transposes back.
"""

from contextlib import ExitStack

import numpy as np

import concourse.bass as bass
import concourse.tile as tile
from concourse import bacc, mybir
from concourse._compat import with_exitstack
from concourse.bass_utils import run_bass_kernel_spmd
from concourse.masks import make_identity

F32 = mybir.dt.float32
I32 = mybir.dt.int32

N_CAT = 26
VOCAB = 50000
D = 128
B = 16384
DENSE = 13
MLP0, MLP1, MLP2 = 512, 256, 128
PROJ = 512
NF = N_CAT + 1              # 27 features entering interaction
GRAM = NF * NF              # 729 full-gram rows
KEXT = MLP2 + GRAM          # 857 projection contraction size
NCORES = 8
BC = B // NCORES            # 2048 samples per core
STRIPES = 4                 # 512-sample stripes per core
SPT = BC // STRIPES         # 512
TILES = 4                   # 128-sample tiles per stripe
TS = 128
NTILES = STRIPES * TILES    # 16
# gram k-tiles: groups of 4 feature-rows (i) -> 108 pair-rows, last has 3 -> 81
GK_ROWS = [108, 108, 108, 108, 108, 108, 81]


@with_exitstack
def _dlrm_kernel(ctx: ExitStack, tc: tile.TileContext,
                 emb, xt, off, w0, w1, w2, wpe, b0, b1, b2, bp, outT):
    nc = tc.nc
    Relu = mybir.ActivationFunctionType.Relu
    Ident = mybir.ActivationFunctionType.Identity
    F32R = mybir.dt.float32r

    def r(ap):
        # PE-native reduced fp32 (FP22): 1 cycle/col when moving N >= 256
        return ap.bitcast(F32R)

    consts = ctx.enter_context(tc.tile_pool(name="consts", bufs=1))
    gp = ctx.enter_context(tc.tile_pool(name="gp", bufs=2))
    ttp = ctx.enter_context(tc.tile_pool(name="ttp", bufs=2))
    zdp = ctx.enter_context(tc.tile_pool(name="zdp", bufs=2))
    h1p = ctx.enter_context(tc.tile_pool(name="h1p", bufs=2))
    h2p = ctx.enter_context(tc.tile_pool(name="h2p", bufs=2))
    motp = ctx.enter_context(tc.tile_pool(name="motp", bufs=2))
    ztp = ctx.enter_context(tc.tile_pool(name="ztp", bufs=2))
    outp = ctx.enter_context(tc.tile_pool(name="outp", bufs=2))
    pm = ctx.enter_context(tc.tile_pool(name="pm", bufs=2, space="PSUM"))
    pt = ctx.enter_context(tc.tile_pool(name="pt", bufs=2, space="PSUM"))
    pg = ctx.enter_context(tc.tile_pool(name="pg", bufs=4, space="PSUM"))

    ident = consts.tile([128, 128], F32)
    make_identity(nc, ident[:])

    w0s = consts.tile([DENSE, MLP0], F32R)
    nc.sync.dma_start(w0s[:], r(w0[:]))
    w1s = consts.tile([128, 4, MLP1], F32)
    for k in range(4):
        nc.sync.dma_start(w1s[:, k, :], w1[k * 128:(k + 1) * 128, :])
    w2s = consts.tile([128, 2, MLP2], F32)
    for k in range(2):
        nc.sync.dma_start(w2s[:, k, :], w2[k * 128:(k + 1) * 128, :])
    # projection weights: k-tile 0 = mlp rows [0:128]; k-tiles 1..7 = gram rows
    wps = consts.tile([128, 8, PROJ], F32R)
    nc.sync.dma_start(wps[:, 0, :], r(wpe[0:128, :]))
    r0 = 128
    for kk in range(7):
        rw0 = GK_ROWS[kk]
        nc.sync.dma_start(wps[0:rw0, kk + 1, :], r(wpe[r0:r0 + rw0, :]))
        r0 += rw0
    b0s = consts.tile([128, 4], F32)
    for m in range(4):
        nc.sync.dma_start(b0s[:, m:m + 1], b0[m * 128:(m + 1) * 128, :])
    b1s = consts.tile([128, 2], F32)
    for m in range(2):
        nc.sync.dma_start(b1s[:, m:m + 1], b1[m * 128:(m + 1) * 128, :])
    b2s = consts.tile([128, 1], F32)
    nc.sync.dma_start(b2s[:], b2[:])
    bps = consts.tile([128, 4], F32)
    for m in range(4):
        nc.sync.dma_start(bps[:, m:m + 1], bp[m * 128:(m + 1) * 128, :])
    xts = consts.tile([DENSE, BC], F32R)
    nc.sync.dma_start(xts[:], r(xt[:]))
    # ALL gather offsets in one load: off[p, kt*26+t]
    offs = consts.tile([128, NTILES * N_CAT], I32)
    nc.sync.dma_start(offs[:], off[:])

    for s in range(STRIPES):
        cs = bass.ds(s * SPT, SPT)
        # ---- bottom MLP (transposed: features on partitions) ----
        h1 = h1p.tile([128, 4, SPT], F32)
        for m in range(4):
            ps = pm.tile([128, SPT], F32)
            nc.tensor.matmul(ps[:], lhsT=w0s[:, m * 128:(m + 1) * 128],
                             rhs=xts[:, cs], start=True, stop=True)
            nc.scalar.activation(h1[:, m, :], ps[:], Relu, bias=b0s[:, m:m + 1])
        h2 = h2p.tile([128, 2, SPT], F32)
        for m in range(2):
            ps = pm.tile([128, SPT], F32)
            for k in range(4):
                nc.tensor.matmul(ps[:], lhsT=w1s[:, k, m * 128:(m + 1) * 128],
                                 rhs=h1[:, k, :], start=(k == 0), stop=(k == 3))
            nc.scalar.activation(h2[:, m, :], ps[:], Relu, bias=b1s[:, m:m + 1])
        mot = motp.tile([128, SPT], F32)
        ps = pm.tile([128, SPT], F32)
        for k in range(2):
            nc.tensor.matmul(ps[:], lhsT=w2s[:, k, :], rhs=h2[:, k, :],
                             start=(k == 0), stop=(k == 1))
        nc.scalar.activation(mot[:], ps[:], Ident, bias=b2s[:, 0:1])

        zt = ztp.tile([108, 7, SPT], F32R)
        for tt in range(TILES):
            kt = s * TILES + tt
            # per-table indirect gathers: g[p, t, :] = emb[offs[p, kt*26+t]]
            g = gp.tile([128, N_CAT, D], F32)
            for t in range(N_CAT):
                c = kt * N_CAT + t
                nc.gpsimd.indirect_dma_start(
                    out=g[:, t, :], out_offset=None, in_=emb[:],
                    in_offset=bass.IndirectOffsetOnAxis(
                        ap=offs[:, c:c + 1], axis=0))
            # T^T layout: [d, s_local, f] with f padded to 32 so gram diag
            # blocks land on 32-aligned PSUM partitions (walrus requirement)
            ttile = ttp.tile([128, TS, 32], F32)
            nc.vector.memset(ttile[:, :, NF:32], 0.0)
            nc.vector.tensor_copy(ttile[:, :, 0], mot[:, tt * TS:(tt + 1) * TS])
            for t in range(N_CAT):
                pst = pt.tile([128, 128], F32)
                nc.tensor.transpose(pst[:], g[:, t, :], ident[:])
                if t % 2 == 0:
                    nc.vector.tensor_copy(ttile[:, :, t + 1], pst[:])
                else:
                    nc.scalar.copy(ttile[:, :, t + 1], pst[:])
            zd = zdp.tile([NF, NF, TS], F32)
            for gi in range(TS // 4):
                pgr = pg.tile([128, 128], F32)
                nc.tensor.matmul(pgr[:], lhsT=ttile[:, gi * 4:(gi + 1) * 4, :],
                                 rhs=ttile[:, gi * 4:(gi + 1) * 4, :],
                                 start=True, stop=True)
                for u in range(4):
                    sl = gi * 4 + u
                    blk = pgr[32 * u:32 * u + 27, 32 * u:32 * u + 27]
                    if u % 4 != 3:
                        nc.vector.tensor_copy(zd[:, :, sl], blk)
                    else:
                        nc.scalar.copy(zd[:, :, sl], blk)
            # re-tile: pair q = i*27+j -> k-tile i//4, row (i%4)*27+j
            for i in range(NF):
                kk = i // 4
                rr = (i % 4) * 27
                nc.sync.dma_start(zt[rr:rr + 27, kk, tt * TS:(tt + 1) * TS],
                                  r(zd[i:i + 1, :, :]))
        # ---- projection ----
        for m in range(4):
            ps = pm.tile([128, SPT], F32)
            nc.tensor.matmul(ps[:], lhsT=wps[:, 0, m * 128:(m + 1) * 128].bitcast(F32),
                             rhs=mot[:], start=True, stop=False)
            for kk in range(7):
                rw = GK_ROWS[kk]
                nc.tensor.matmul(ps[:], lhsT=wps[0:rw, kk + 1, m * 128:(m + 1) * 128],
                                 rhs=zt[0:rw, kk, :], start=False, stop=(kk == 6))
            ot = outp.tile([128, SPT], F32)
            nc.scalar.activation(ot[:], ps[:], Ident, bias=bps[:, m:m + 1])
            nc.sync.dma_start(outT[m * 128:(m + 1) * 128, cs], ot[:])


_PROG = None


def _build_program():
    global _PROG
    if _PROG is not None:
        return _PROG
    nc = bacc.Bacc("TRN2", target_bir_lowering=False, debug=False,
                   enable_asserts=False, num_devices=NCORES)
    emb = nc.dram_tensor("emb", [N_CAT * VOCAB, D], F32, kind="ExternalInput").ap()
    xt = nc.dram_tensor("xt", [DENSE, BC], F32, kind="ExternalInput").ap()
    off = nc.dram_tensor("off", [128, NTILES * N_CAT], I32,
                         kind="ExternalInput").ap()
    w0 = nc.dram_tensor("w0", [DENSE, MLP0], F32, kind="ExternalInput").ap()
    w1 = nc.dram_tensor("w1", [MLP0, MLP1], F32, kind="ExternalInput").ap()
    w2 = nc.dram_tensor("w2", [MLP1, MLP2], F32, kind="ExternalInput").ap()
    wpe = nc.dram_tensor("wpe", [KEXT, PROJ], F32, kind="ExternalInput").ap()
    b0 = nc.dram_tensor("b0", [MLP0, 1], F32, kind="ExternalInput").ap()
    b1 = nc.dram_tensor("b1", [MLP1, 1], F32, kind="ExternalInput").ap()
    b2 = nc.dram_tensor("b2", [MLP2, 1], F32, kind="ExternalInput").ap()
    bp = nc.dram_tensor("bp", [PROJ, 1], F32, kind="ExternalInput").ap()
    outT = nc.dram_tensor("outT", [PROJ, BC], F32, kind="ExternalOutput").ap()
    with tile.TileContext(nc) as tc:
        _dlrm_kernel(tc, emb, xt, off, w0, w1, w2, wpe, b0, b1, b2, bp, outT)
    nc.compile()
    _PROG = nc
    return nc


def _expand_wp(Wp: np.ndarray) -> np.ndarray:
    """[479, 512] -> [857, 512]: full-gram rows, 0.5 off-diag, 0 diag."""
    wpe = np.zeros((KEXT, PROJ), np.float32)
    wpe[:MLP2] = Wp[:MLP2]
    row, col = np.triu_indices(NF, k=1)
    pair_q = {}
    for q, (i, j) in enumerate(zip(row, col)):
        pair_q[(i, j)] = q
    for i in range(NF):
        for j in range(NF):
            if i == j:
                continue
            a, b = (i, j) if i < j else (j, i)
            wpe[MLP2 + i * NF + j] = 0.5 * Wp[MLP2 + pair_q[(a, b)]]
    return wpe


def prepare_in_maps(dense, emb_indices, W0, b0, W1, b1, W2, b2, emb_tables, Wp, bp):
    dense = np.asarray(dense, np.float32)
    emb_indices = np.asarray(emb_indices)
    emb_flat = np.ascontiguousarray(np.asarray(emb_tables, np.float32)
                                    .reshape(N_CAT * VOCAB, D))
    wpe = _expand_wp(np.asarray(Wp, np.float32))
    base = (np.arange(N_CAT, dtype=np.int64) * VOCAB)[:, None]
    gidx = (emb_indices.astype(np.int64) + base).astype(np.int32)  # [26, B]
    common = {
        "emb": emb_flat,
        "w0": np.asarray(W0, np.float32),
        "w1": np.asarray(W1, np.float32),
        "w2": np.asarray(W2, np.float32),
        "wpe": wpe,
        "b0": np.asarray(b0, np.float32).reshape(MLP0, 1),
        "b1": np.asarray(b1, np.float32).reshape(MLP1, 1),
        "b2": np.asarray(b2, np.float32).reshape(MLP2, 1),
        "bp": np.asarray(bp, np.float32).reshape(PROJ, 1),
    }
    in_maps = []
    for c in range(NCORES):
        sl = slice(c * BC, (c + 1) * BC)
        # off[p, kt*26+t] = gidx[t, c*2048 + kt*128 + p]
        oc = gidx[:, sl].reshape(N_CAT, NTILES, TS)  # [t, kt, p]
        off = np.ascontiguousarray(oc.transpose(2, 1, 0)
                                   .reshape(TS, NTILES * N_CAT))
        in_maps.append(dict(common,
                            xt=np.ascontiguousarray(dense[sl].T),
                            off=off))
    return in_maps


def kernel(**inputs) -> np.ndarray:
    nc = _build_program()
    in_maps = prepare_in_maps(**inputs)
    res = run_bass_kernel_spmd(nc, in_maps, list(range(NCORES)))
    out = np.empty((B, PROJ), np.float32)
    for c in range(NCORES):
        out[c * BC:(c + 1) * BC] = res.results[c]["outT"].T
    return out
